# revision 10
# baseline (speedup 1.0000x reference)
"""DeepFM forward on 8 Trainium2 NeuronCores (Bass/Tile).

Strategy: data-parallel over batch (2048 samples/core), embedding tables
replicated. Tables are host-packed into 64B rows [emb2 as bf16 x16 | emb1
f32 | pad] so each embedding lookup is one aligned 64B descriptor of a
single chunked SWDGE indirect-DMA gather. The DNN runs in bf16 on the PE
(fp32 PSUM accumulation); the FM linear terms stay fp32. Per-core sample
mapping is b_local = p*16 + t (partition p, sample-tile t) so the final
store is 128 contiguous 64B runs.
"""

import sys

if "/opt/trn_rl_repo" not in sys.path:
    sys.path.insert(0, "/opt/trn_rl_repo")

import numpy as np
import ml_dtypes

import concourse.bass as bass
import concourse.bacc as bacc
import concourse.mybir as mybir
import concourse.tile as tile
import concourse.bass_utils as bass_utils

BF16 = ml_dtypes.bfloat16
DT = mybir.dt
AF = mybir.ActivationFunctionType
OP = mybir.AluOpType
AX = mybir.AxisListType

B, NS, ND, V, E, H = 16384, 26, 13, 100000, 16, 400
N_CORES = 8
B_CORE = B // N_CORES  # 2048
P = 128
T = B_CORE // P        # 16 sample tiles per core
ROW_W = 16             # f32 words per packed table row (64B)
NROWS = T * NS         # 416 gathered rows per partition
NB = B_CORE // 512     # 4 sample blocks of 512 for L0/L1
GATHER_SPLITS = 8
K0 = [128, 128, 128, 45]   # dnn_in k-chunks (416 e2 + 13 dense)
K1 = [128, 128, 128, 16]   # h k-chunks (400)
K2 = [128, 128, 128, 17]   # h k-chunks + ones row (b2 fold)

_PROG = None


def _build_program():
    f32, bf16, i32 = DT.float32, DT.bfloat16, DT.int32
    nc = bacc.Bacc(
        "TRN2", target_bir_lowering=False, debug=False, num_devices=N_CORES
    )

    tab = nc.dram_tensor("tab", [NS * V, ROW_W], f32, kind="ExternalInput").ap()
    gidx = nc.dram_tensor("gidx", [P, NROWS], i32, kind="ExternalInput").ap()
    xd = nc.dram_tensor("xd", [P, T * ND], f32, kind="ExternalInput").ap()
    xdt = nc.dram_tensor("xdt", [ND, B_CORE], bf16, kind="ExternalInput").ap()
    w0d = [
        nc.dram_tensor(f"w0_{c}", [sz, H], bf16, kind="ExternalInput").ap()
        for c, sz in enumerate(K0)
    ]
    w1d = [
        nc.dram_tensor(f"w1_{c}", [sz, H], bf16, kind="ExternalInput").ap()
        for c, sz in enumerate(K1)
    ]
    w2d = [
        nc.dram_tensor(f"w2_{c}", [sz, H], bf16, kind="ExternalInput").ap()
        for c, sz in enumerate(K2)
    ]
    woutd = nc.dram_tensor("woutb", [P, H], bf16, kind="ExternalInput").ap()
    b0d = nc.dram_tensor("b0p", [P, 4], f32, kind="ExternalInput").ap()
    b1d = nc.dram_tensor("b1p", [P, 4], f32, kind="ExternalInput").ap()
    linwd = nc.dram_tensor("linwb", [P, ND], f32, kind="ExternalInput").ap()
    biasd = nc.dram_tensor("biasb", [P, 1], f32, kind="ExternalInput").ap()
    identd = nc.dram_tensor("identb", [P, P], bf16, kind="ExternalInput").ap()
    onesd = nc.dram_tensor("onesb", [1, B_CORE], bf16, kind="ExternalInput").ap()
    out = nc.dram_tensor("out", [B_CORE], f32, kind="ExternalOutput").ap()

    with tile.TileContext(nc) as tc:
        with (
            tc.tile_pool(name="const", bufs=1) as cp,
            tc.tile_pool(name="work", bufs=2) as wp,
            tc.tile_pool(name="psA", bufs=2, space="PSUM") as pA,
            tc.tile_pool(name="psB", bufs=1, space="PSUM") as pB,
            tc.tile_pool(name="psL", bufs=3, space="PSUM") as pL,
            tc.tile_pool(name="ps2", bufs=2, space="PSUM") as p2,
        ):
            # ---- index load + gather (keep Pool queue gather-only) ----
            # HW indirect DMA consumes ONE offset per partition per
            # instruction (run of dest-free words from table[idx[p]]), so
            # each instruction fetches 128 rows: slot (p, j) <- tab[idx[p,j]]
            idxt = cp.tile([P, NROWS], i32)
            nc.sync.dma_start(out=idxt[:], in_=gidx[:])
            g = cp.tile([P, NROWS * ROW_W], f32)
            for j in range(NROWS):
                nc.gpsimd.indirect_dma_start(
                    out=g[:, j * ROW_W : (j + 1) * ROW_W],
                    out_offset=None,
                    in_=tab[:],
                    in_offset=bass.IndirectOffsetOnAxis(
                        ap=idxt[:, j : j + 1], axis=0
                    ),
                )

            # ---- constants ----
            ident = cp.tile([P, P], bf16)
            nc.sync.dma_start(out=ident[:], in_=identd[:])
            xds = cp.tile([P, T * ND], f32)
            nc.sync.dma_start(out=xds[:], in_=xd[:])
            w0t = [cp.tile([sz, H], bf16, tag=f"w0_{c}", name=f"w0t_{c}") for c, sz in enumerate(K0)]
            w1t = [cp.tile([sz, H], bf16, tag=f"w1_{c}", name=f"w1t_{c}") for c, sz in enumerate(K1)]
            w2t = [cp.tile([sz, H], bf16, tag=f"w2_{c}", name=f"w2t_{c}") for c, sz in enumerate(K2)]
            for tl, d in zip(w0t + w1t + w2t, w0d + w1d + w2d):
                nc.sync.dma_start(out=tl[:], in_=d[:])
            woutt = cp.tile([P, H], bf16)
            nc.sync.dma_start(out=woutt[:], in_=woutd[:])
            b0t = cp.tile([P, 4], f32)
            nc.sync.dma_start(out=b0t[:], in_=b0d[:])
            b1t = cp.tile([P, 4], f32)
            nc.sync.dma_start(out=b1t[:], in_=b1d[:])
            linwt = cp.tile([P, ND], f32)
            nc.sync.dma_start(out=linwt[:], in_=linwd[:])
            biast = cp.tile([P, 1], f32)
            nc.sync.dma_start(out=biast[:], in_=biasd[:])

            # ---- activations (transposed, k-chunked) ----
            dnnT = cp.tile([P, 3, B_CORE], bf16)
            dnnT3 = cp.tile([45, B_CORE], bf16)
            nc.sync.dma_start(out=dnnT3[32:45, :], in_=xdt[:])
            h0T = cp.tile([P, 3, B_CORE], bf16)
            h0T3 = cp.tile([16, B_CORE], bf16)
            h1T = cp.tile([P, 3, B_CORE], bf16)
            h1T3 = cp.tile([17, B_CORE], bf16)
            nc.sync.dma_start(out=h1T3[16:17, :], in_=onesd[:])

            # ---- FM / linear accumulators (column t per sample tile) ----
            crossb = cp.tile([P, T], f32)
            ssb = cp.tile([P, T], f32)
            e1b = cp.tile([P, T], f32)
            lindb = cp.tile([P, T], f32)
            yb = cp.tile([P, T], f32)

            g_bf = g[:].bitcast(bf16).rearrange(
                "p (t f u) -> p t f u", t=T, f=NS
            )  # u = 32 bf16 units/row; e2 at u[0:16]
            g_f32 = g[:].rearrange("p (t f w) -> p t f w", t=T, f=NS)

            # compact e2 out of the padded gathered rows (matmul operands
            # need single-free-dim APs); alternate DVE/ACT per split
            e2c = cp.tile([P, T * NS * E], bf16)
            tps = T // GATHER_SPLITS  # sample tiles per gather split
            for s in range(GATHER_SPLITS):
                src = g_bf[:, s * tps : (s + 1) * tps, :, 0:16]
                dst = e2c[
                    :, s * tps * NS * E : (s + 1) * tps * NS * E
                ].rearrange("p (t f e) -> p t f e", t=tps, f=NS)
                if s % 2 == 0:
                    nc.vector.tensor_copy(out=dst, in_=src)
                else:
                    nc.scalar.copy(out=dst, in_=src)
            e2c_t = e2c[:].rearrange("p (t u) -> p t u", t=T)
            e2c_r = e2c[:].rearrange("p (t f e) -> p t e f", t=T, f=NS)

            def emit_tile_front(t):
                """Transposes + FM for sample tile t."""
                psA_t = pA.tile([P, 3 * P], bf16, tag="psA")
                for c in range(3):
                    nc.tensor.transpose(
                        out=psA_t[:, c * P : (c + 1) * P],
                        in_=e2c_t[:, t, c * P : (c + 1) * P],
                        identity=ident[:],
                    )
                psB_t = pB.tile([32, P], bf16, tag="psB")
                nc.tensor.transpose(
                    out=psB_t[:], in_=e2c_t[:, t, 384:416], identity=ident[:]
                )
                nc.vector.tensor_copy(
                    out=dnnT[:, :, t * P : (t + 1) * P],
                    in_=psA_t[:].rearrange("p (c n) -> p c n", c=3),
                )
                nc.scalar.copy(out=dnnT3[0:32, t * P : (t + 1) * P], in_=psB_t[:])
                # FM second-order pieces
                s2t = wp.tile([P, E], f32, tag="s2")
                nc.vector.reduce_sum(out=s2t[:], in_=e2c_r[:, t], axis=AX.X)
                sq16 = wp.tile([P, E], f32, tag="sq16")
                nc.scalar.activation(
                    out=sq16[:], in_=s2t[:], func=AF.Square,
                    accum_out=crossb[:, t : t + 1],
                )
                sqe = wp.tile([P, NS * E], bf16, tag="sqe")
                nc.scalar.activation(
                    out=sqe[:], in_=e2c_t[:, t, :], func=AF.Square,
                    accum_out=ssb[:, t : t + 1],
                )
                # linear terms (fp32)
                nc.vector.reduce_sum(
                    out=e1b[:, t : t + 1], in_=g_f32[:, t, :, 8], axis=AX.X
                )
                sc13 = wp.tile([P, ND], f32, tag="sc13")
                nc.vector.tensor_tensor_reduce(
                    out=sc13[:],
                    in0=xds[:, t * ND : (t + 1) * ND],
                    in1=linwt[:],
                    scale=1.0,
                    scalar=0.0,
                    op0=OP.mult,
                    op1=OP.add,
                    accum_out=lindb[:, t : t + 1],
                )

            def emit_l01(nb, wts, bt, src, src3, dst, dst3, dst3_rows):
                """One 512-sample block of an h-major layer (L0 or L1)."""
                ns = slice(nb * 512, (nb + 1) * 512)
                for hc in range(4):
                    m = 128 if hc < 3 else 16
                    ps = pL.tile([m, 512], f32, tag="psL")
                    for kc in range(4):
                        rhs = src[:, kc, ns] if kc < 3 else src3[:, ns]
                        nc.tensor.matmul(
                            ps[:],
                            wts[kc][:, hc * 128 : hc * 128 + m],
                            rhs,
                            start=(kc == 0),
                            stop=(kc == 3),
                        )
                    dst_ap = dst[:, hc, ns] if hc < 3 else dst3[0:dst3_rows, ns]
                    nc.scalar.activation(
                        out=dst_ap, in_=ps[:], func=AF.Relu,
                        bias=bt[0:m, hc : hc + 1],
                    )

            def emit_l2(t):
                """Sample-major last hidden layer + output dot for tile t."""
                ts = slice(t * P, (t + 1) * P)
                ps2 = p2.tile([P, H], f32, tag="ps2")
                for kc in range(4):
                    lhsT = h1T[:, kc, ts] if kc < 3 else h1T3[:, ts]
                    nc.tensor.matmul(
                        ps2[:], lhsT, w2t[kc][:], start=(kc == 0), stop=(kc == 3)
                    )
                h2s = wp.tile([P, H], bf16, tag="h2s")
                nc.scalar.activation(out=h2s[:], in_=ps2[:], func=AF.Relu)
                scrH = wp.tile([P, H], bf16, tag="scrH")
                nc.vector.tensor_tensor_reduce(
                    out=scrH[:],
                    in0=h2s[:],
                    in1=woutt[:],
                    scale=1.0,
                    scalar=0.0,
                    op0=OP.mult,
                    op1=OP.add,
                    accum_out=yb[:, t : t + 1],
                )

            # software-pipelined emission: L1/L2 of block nb-1 interleave
            # with L0 of block nb so ACT relu-copies hide under PE matmuls
            dnn_src = (dnnT[:], dnnT3[:])
            for nb in range(NB):
                for t in range(nb * 4, nb * 4 + 4):
                    emit_tile_front(t)
                emit_l01(nb, w0t, b0t, *dnn_src, h0T[:], h0T3[:], 16)
                if nb > 0:
                    emit_l01(nb - 1, w1t, b1t, h0T[:], h0T3[:], h1T[:], h1T3[:], 16)
                    for t in range((nb - 1) * 4, nb * 4):
                        emit_l2(t)
            emit_l01(NB - 1, w1t, b1t, h0T[:], h0T3[:], h1T[:], h1T3[:], 16)
            for t in range((NB - 1) * 4, NB * 4):
                emit_l2(t)

            # ---- final assembly:
            # logit = e1 + lin_dense + 0.5*(cross - ss) + y + bias ----
            t1 = cp.tile([P, T], f32)
            t2 = cp.tile([P, T], f32)
            logit = cp.tile([P, T], f32)
            nc.vector.tensor_tensor(out=t1[:], in0=e1b[:], in1=lindb[:], op=OP.add)
            nc.vector.tensor_tensor(out=t2[:], in0=crossb[:], in1=ssb[:], op=OP.subtract)
            nc.vector.tensor_scalar_mul(out=t2[:], in0=t2[:], scalar1=0.5)
            nc.vector.tensor_tensor(out=t1[:], in0=t1[:], in1=t2[:], op=OP.add)
            nc.vector.tensor_tensor(out=t1[:], in0=t1[:], in1=yb[:], op=OP.add)
            nc.vector.tensor_scalar_add(out=logit[:], in0=t1[:], scalar1=biast[:, 0:1])
            nc.sync.dma_start(
                out=out.rearrange("(p t) -> p t", p=P), in_=logit[:]
            )

    nc.compile()
    return nc


def get_program():
    global _PROG
    if _PROG is None:
        _PROG = _build_program()
    return _PROG


def make_in_maps(inputs):
    X_sparse = np.asarray(inputs["X_sparse"], np.int32)
    X_dense = np.asarray(inputs["X_dense"], np.float32)
    emb1 = np.asarray(inputs["emb1"], np.float32)
    emb2 = np.asarray(inputs["emb2"], np.float32)
    lin_weight = np.asarray(inputs["lin_weight"], np.float32)
    bias = np.asarray(inputs["bias"], np.float32)
    W0 = np.asarray(inputs["W0"], np.float32)
    b0 = np.asarray(inputs["b0"], np.float32)
    W1 = np.asarray(inputs["W1"], np.float32)
    b1 = np.asarray(inputs["b1"], np.float32)
    W2 = np.asarray(inputs["W2"], np.float32)
    b2 = np.asarray(inputs["b2"], np.float32)
    W_out = np.asarray(inputs["W_out"], np.float32)

    # packed table: 64B rows = [emb2 bf16 x16 | emb1 f32 | 28B pad]
    tab = np.zeros((NS * V, ROW_W), np.float32)
    e2b = emb2.astype(BF16)
    tab.view(np.uint16).reshape(NS * V, 2 * ROW_W)[:, 0:16] = (
        e2b.view(np.uint16).reshape(NS * V, 16)
    )
    tab[:, 8] = emb1.reshape(NS * V)

    g_all = X_sparse + (np.arange(NS, dtype=np.int32) * V)[None, :]

    def padk(w, chunks):
        ksz = sum(chunks)
        if w.shape[0] < ksz:
            w = np.vstack([w, np.zeros((ksz - w.shape[0], w.shape[1]), w.dtype)])
        outs, o = [], 0
        for c in chunks:
            outs.append(np.ascontiguousarray(w[o : o + c]).astype(BF16))
            o += c
        return outs

    w0c = padk(W0, K0)
    w1c = padk(W1, K1)
    w2c = padk(np.vstack([W2, b2[None, :]]), K2)

    def padb(b):
        bp = np.zeros(512, np.float32)
        bp[:H] = b
        return np.ascontiguousarray(bp.reshape(4, P).T)

    shared = {
        "tab": tab,
        "w0_0": w0c[0], "w0_1": w0c[1], "w0_2": w0c[2], "w0_3": w0c[3],
        "w1_0": w1c[0], "w1_1": w1c[1], "w1_2": w1c[2], "w1_3": w1c[3],
        "w2_0": w2c[0], "w2_1": w2c[1], "w2_2": w2c[2], "w2_3": w2c[3],
        "woutb": np.ascontiguousarray(
            np.broadcast_to(W_out[:, 0][None, :], (P, H))
        ).astype(BF16),
        "b0p": padb(b0),
        "b1p": padb(b1),
        "linwb": np.ascontiguousarray(
            np.broadcast_to(lin_weight[:, 0][None, :], (P, ND))
        ),
        "biasb": np.full((P, 1), bias[0], np.float32),
        "identb": np.eye(P, dtype=BF16),
        "onesb": np.ones((1, B_CORE), BF16),
    }

    in_maps = []
    for c in range(N_CORES):
        rows = slice(c * B_CORE, (c + 1) * B_CORE)
        xc = X_dense[rows].reshape(P, T, ND)
        in_maps.append(
            dict(
                shared,
                gidx=np.ascontiguousarray(g_all[rows].reshape(P, NROWS)),
                xd=np.ascontiguousarray(xc.reshape(P, T * ND)),
                xdt=np.ascontiguousarray(
                    xc.transpose(2, 1, 0).reshape(ND, B_CORE)
                ).astype(BF16),
            )
        )
    return in_maps


def kernel(**inputs) -> np.ndarray:
    nc = get_program()
    in_maps = make_in_maps(inputs)
    res = bass_utils.run_bass_kernel_spmd(
        nc, in_maps, core_ids=list(range(N_CORES))
    )
    return np.concatenate(
        [res.results[c]["out"] for c in range(N_CORES)]
    ).reshape(B, 1)
